# revision 50
# baseline (speedup 1.0000x reference)
"""ACmix Trainium2 kernel: batch-parallel over 8 NeuronCores.

Per-core graph (one batch element, x: (256, 64, 64)):
  - qkv 1x1 conv as PE matmul; the grouped kernel-generator is folded through
    the qkv weights on the host (alpha * w_kg_g @ w_qkv_grouped) and rides as
    one extra 128-row M tile. q/k tiles are emitted first so the logits
    pipeline starts while v/kg tiles are still computing.
  - 3x3 window shifts are free-axis AP offsets into zero-padded (guard-band)
    k/v buffers; an odd-offset copy of k keeps DVE product reads pair-aligned
    (2x mode).
  - logits: DVE/GpSimd shifted elementwise products + PE selector matmuls
    reducing over d into (38, 512) PSUM tiles (coefficient rows are grouped
    by dj so image-edge fixups hit 32-aligned partition ranges).
  - softmax: edge columns of expS forced to exp(0)=1 (reference semantics
    for out-of-image shifts), Z via a replicating PE matmul, 1/Z via ActE
    ln + exp(-x + ln(beta)).
  - combined coefficients wc = invzb*expS + alpha*kg (wide in-place 2x DVE
    ops); wc edge columns zeroed so edge-wrapped pv terms vanish.
  - V-contraction: wc is bounced to DRAM and broadcast-read back replicated
    across each head's 64 d partitions (bf16 SBUF), multiplied by shifted v
    in place on DVE at 2x, and accumulated over the 9 shifts by PE identity
    matmuls in PSUM. This removes the PE replication matmuls and keeps the
    multiplies off the PSUM-read (1x) path.
  - final 1x1 proj matmul, f32 DMA out.

The whole kernel is software-pipelined at 2048-column granularity across the
two head-pairs so the broadcast DMA traffic spreads over the full runtime and
PE stays continuously busy (high p-state).
"""

import os
import sys

import numpy as np

sys.path.insert(0, "/opt/trn_rl_repo")

H_IMG = 64
W_IMG = 64
L = H_IMG * W_IMG  # 4096
C = 256
NH = 4
D = 64
K2 = 9
NCORES = 8
NT = 8  # 512-column chunks

LAST_EXEC_NS = None
LAST_TRACE_DIR = None

# shift table: s = 3*(di+1) + (dj+1), flat offset 64*di + dj
SHIFTS = [(di, dj) for di in (-1, 0, 1) for dj in (-1, 0, 1)]
# coefficient-row order: group shifts by dj so edge fixes hit contiguous,
# 32-aligned partition ranges: dj=-1 -> rows 0-5, dj=0 -> rows 6-11,
# dj=+1 -> rows 32-37. Rows 12-31 are unused (engines require 32-aligned
# partition bases for sub-tile views).
DJORD = {0: 0, 3: 1, 6: 2, 1: 3, 4: 4, 7: 5, 2: 6, 5: 7, 8: 8}
RCO = 38  # coefficient tile height


def ROWOF(s, hl):
    g = DJORD[s]
    return g * 2 + hl if g < 6 else 32 + (g - 6) * 2 + hl


def _build_graph(alpha, beta):
    import concourse.bacc as bacc
    import concourse.mybir as mybir
    from concourse import tile

    f32 = mybir.dt.float32
    bf16 = mybir.dt.bfloat16
    MULT = mybir.AluOpType.mult
    ADD = mybir.AluOpType.add
    EXP = mybir.ActivationFunctionType.Exp
    LOG = mybir.ActivationFunctionType.Ln
    COPY = mybir.ActivationFunctionType.Copy
    IDENT = mybir.ActivationFunctionType.Identity

    nc = bacc.Bacc(None, target_bir_lowering=False)

    x_ext = nc.declare_dram_parameter("x", [C, L], f32, isOutput=False)
    # cst0: wt0|wpt0|osel|ident, cst1: wt1|wpt1|ozrep(pad128)|bkg
    W0 = 896 + 256 + 9 * RCO + 128
    W1 = 896 + 256 + RCO + 1
    cst0_ext = nc.declare_dram_parameter("cst0", [128, W0], bf16, isOutput=False)
    cst1_ext = nc.declare_dram_parameter("cst1", [128, W1 - 1], bf16, isOutput=False)
    bkg_ext = nc.declare_dram_parameter("bkgv", [128, 1], f32, isOutput=False)
    out_ext = nc.declare_dram_parameter("out", [C, L], f32, isOutput=True)
    stage = [nc.dram_tensor(f"stage{h}", [RCO, L], bf16, kind="Internal") for h in range(2)]

    lnbeta = float(np.log(beta))

    with tile.TileContext(nc) as tc:
        with (
            tc.tile_pool(name="const", bufs=1) as cpool,
            tc.tile_pool(name="data", bufs=1) as dpool,
            tc.tile_pool(name="prod", bufs=5) as ppool,
            tc.tile_pool(name="lnz", bufs=4) as lnzpool,
            tc.tile_pool(name="rp", bufs=12) as rppool,
            tc.tile_pool(name="outsb", bufs=2) as opool,
            tc.tile_pool(name="ps_mm", bufs=2, space="PSUM") as ps_mm,
            tc.tile_pool(name="ps_lg", bufs=2, space="PSUM") as ps_lg,
            tc.tile_pool(name="ps_acc", bufs=4, space="PSUM") as ps_acc,
        ):
            # ---- constants (two fused blobs; bkg stays f32 via its own
            # small piece of cst1 read at f32) ----
            cst0 = cpool.tile([128, W0], bf16, tag="cst0")
            cst1b = cpool.tile([128, W1 - 1], bf16, tag="cst1b")
            bkg_sb = cpool.tile([128, 1], f32, tag="bkg")
            lnb_sb = cpool.tile([RCO, 1], f32, tag="lnb")
            nc.vector.memset(lnb_sb[:], lnbeta)
            nc.sync.dma_start(cst0[:], cst0_ext[:])
            nc.sync.dma_start(cst1b[:], cst1_ext[:])
            nc.sync.dma_start(bkg_sb[:], bkg_ext[:])
            wt_bf = [cst0[:, 0:896], cst1b[:, 0:896]]
            wpt_bf = [cst0[:, 896:1152], cst1b[:, 896:1152]]
            osel_bf = cst0[:, 1152 : 1152 + 9 * RCO]
            ident_bf = cst0[:, 1152 + 9 * RCO : 1152 + 9 * RCO + 128]
            ozrep_bf = cst1b[0:RCO, 1152 : 1152 + RCO]

            # ---- input (cast f32 -> bf16 during DMA) ----
            x_bf = [dpool.tile([128, L], bf16, tag=f"x{k}", name=f"x{k}") for k in range(2)]
            for nt in range(NT):
                ncol = slice(nt * 512, (nt + 1) * 512)
                for k in range(2):
                    nc.gpsimd.dma_start(x_bf[k][:, ncol], x_ext[128 * k : 128 * (k + 1), ncol])

            # ---- main SBUF tensors ----
            q_bf = [dpool.tile([128, L], bf16, tag=f"q{h}", name=f"q{h}") for h in range(2)]
            GUARD = 66
            FLATW = GUARD + L + GUARD  # 4228
            ke = [dpool.tile([128, FLATW], bf16, tag=f"ke{h}", name=f"ke{h}") for h in range(2)]
            ko = [dpool.tile([128, FLATW], bf16, tag=f"ko{h}", name=f"ko{h}") for h in range(2)]
            ve = [dpool.tile([128, FLATW], bf16, tag=f"ve{h}", name=f"ve{h}") for h in range(2)]
            kg = [dpool.tile([RCO, L], bf16, tag=f"kg{h}", name=f"kg{h}") for h in range(2)]
            expS = [dpool.tile([RCO, L], bf16, tag=f"expS{h}", name=f"expS{h}") for h in range(2)]

            # acc reuses q_bf: q is dead once its head-pair's products are done
            acc = q_bf

            # zero the guard bands
            for t in ko:
                nc.vector.memset(t[:, 0 : GUARD + 1], 0.0)
                nc.vector.memset(t[:, GUARD + 1 + L : FLATW], 0.0)
            for t in ke + ve:
                nc.vector.memset(t[:, 0:GUARD], 0.0)
                nc.vector.memset(t[:, GUARD + L : FLATW], 0.0)

            # ---- phase B: qkv + kernel-gen matmul, evictions ----
            # split: q/k tiles first (unblock phase C), v/kg tiles later
            def emit_B(tiles):
                for nt in range(NT):
                    ncol = slice(nt * 512, (nt + 1) * 512)
                    for mi, m0, msz in tiles:
                        ps = ps_mm.tile([msz, 512], f32, tag="mmps", name="qkvps", padded_shape=[128, 512])
                        for kt in range(2):
                            nc.tensor.matmul(
                                ps[:],
                                wt_bf[kt][:, m0 : m0 + msz],
                                x_bf[kt][:, ncol],
                                start=(kt == 0),
                                stop=(kt == 1),
                            )
                        if mi < 2:
                            nc.scalar.activation(q_bf[mi][:, ncol], ps[:], COPY)
                        elif mi < 4:
                            hp = mi - 2
                            dst = ke[hp][:, GUARD + 512 * nt : GUARD + 512 * (nt + 1)]
                            nc.scalar.activation(dst, ps[:], COPY)
                        elif mi < 6:
                            hp = mi - 4
                            dst = ve[hp][:, GUARD + 512 * nt : GUARD + 512 * (nt + 1)]
                            nc.scalar.activation(dst, ps[:], COPY)
                        else:
                            for hp in range(2):
                                nc.scalar.activation(
                                    kg[hp][:, ncol],
                                    ps[64 * hp : 64 * hp + RCO, :],
                                    IDENT,
                                    bias=bkg_sb[64 * hp : 64 * hp + RCO, :],
                                )
                    # odd-offset k copy, chunked so C can start early
                    if tiles[0][0] == 0:
                        for hp in range(2):
                            c0 = GUARD + 512 * nt
                            nc.vector.tensor_copy(
                                ko[hp][:, c0 + 1 : c0 + 513],
                                ke[hp][:, c0 : c0 + 512],
                            )

            def k_view(hp, s, l0, ncols):
                di, dj = SHIFTS[s]
                off = 64 * di + dj
                if (GUARD + off) % 2 == 0:
                    base = GUARD + off + l0
                    return ke[hp][:, base : base + ncols]
                base = GUARD + 1 + off + l0
                return ko[hp][:, base : base + ncols]

            def v_view(hp, s, l0, ncols):
                di, dj = SHIFTS[s]
                off = 64 * di + dj
                base = GUARD + off + l0
                return ve[hp][:, base : base + ncols]

            def coeff_edge_fix(t, value):
                # t: (18, L) tensor in dj-grouped row order; set invalid-dj
                # entries to `value`: rows 0:6 have dj=-1 (invalid at image
                # col 0), rows 12:18 have dj=+1 (invalid at image col 63)
                v3 = t[:].rearrange("p (r c) -> p r c", c=64)
                nc.vector.memset(v3[0:6, :, 0:1], value)
                nc.vector.memset(v3[32:38, :, 63:64], value)

            # ---- phase emitters (C/D/E/F), software-pipelined across the
            # two head-pairs so DMA-paced E overlaps compute-paced C/D ----
            def emit_C(hp, g):
                # logits products + selector matmuls + exp for one 1024-chunk
                gc = slice(g * 1024, (g + 1) * 1024)
                qv = q_bf[hp][:, gc]
                lgs = [ps_lg.tile([RCO, 512], f32, tag="lg", name="lg") for _ in range(2)]
                for s in range(K2):
                    prod = ppool.tile([128, 1024], bf16, tag="prod", name="prod")
                    kap = k_view(hp, s, g * 1024, 1024)
                    eng = nc.gpsimd if s in (1, 7) else nc.vector
                    eng.tensor_tensor(prod[:], qv, kap, MULT)
                    for j in range(2):
                        nc.tensor.matmul(
                            lgs[j][:],
                            osel_bf[:, s * RCO : (s + 1) * RCO],
                            prod[:, j * 512 : (j + 1) * 512],
                            start=(s == 0),
                            stop=(s == K2 - 1),
                        )
                for j in range(2):
                    ncol = slice(g * 1024 + j * 512, g * 1024 + (j + 1) * 512)
                    nc.scalar.activation(expS[hp][:, ncol], lgs[j][:], EXP)

            lnz_tiles = {}

            def emit_D_z(hp, gb):
                # dj-edge logits are garbage (wrapped rows); reference has
                # logit=0 there -> exp=1 must enter Z (rows of this half)
                v3 = expS[hp][:].rearrange("p (r c) -> p r c", c=64)
                r0, r1 = gb * 32, (gb + 1) * 32
                nc.gpsimd.memset(v3[0:6, r0:r1, 0:1], 1.0)
                nc.gpsimd.memset(v3[32:38, r0:r1, 63:64], 1.0)
                lz = lnzpool.tile([RCO, 2048], bf16, tag="lnz", name="lnz")
                lnz_tiles[(hp, gb)] = lz
                for k4 in range(4):
                    nt = 4 * gb + k4
                    ncol = slice(nt * 512, (nt + 1) * 512)
                    zr = ps_mm.tile([RCO, 512], f32, tag="mmps", name="zr", padded_shape=[128, 512])
                    nc.tensor.matmul(zr[:], ozrep_bf[:], expS[hp][:, ncol])
                    nc.scalar.activation(lz[:, k4 * 512 : (k4 + 1) * 512], zr[:], LOG)

            def emit_D_wc(hp, gb):
                # invzb = beta * exp(-ln Z); wc = invzb * expS + kg (in expS)
                hc = slice(gb * 2048, (gb + 1) * 2048)
                lz = lnz_tiles.pop((hp, gb))
                iz = lnzpool.tile([RCO, 2048], bf16, tag="lnz", name="invz")
                nc.scalar.activation(iz[:], lz[:], EXP, scale=-1.0, bias=lnb_sb[:])
                nc.vector.tensor_tensor(
                    expS[hp][:, hc], iz[:], expS[hp][:, hc], MULT
                )
                nc.vector.tensor_tensor(
                    expS[hp][:, hc], expS[hp][:, hc], kg[hp][:, hc], ADD
                )
                # edge-wrapped pv terms must vanish (rows of this half only)
                v3 = expS[hp][:].rearrange("p (r c) -> p r c", c=64)
                r0, r1 = gb * 32, (gb + 1) * 32
                nc.gpsimd.memset(v3[0:6, r0:r1, 0:1], 0.0)
                nc.gpsimd.memset(v3[32:38, r0:r1, 63:64], 0.0)
                # bounce this half of wc to DRAM for phase-E broadcast reads
                nc.scalar.dma_start(stage[hp][:, hc], expS[hp][:, hc])

            # phase E: wc broadcast-read back from DRAM replicated across
            # each head's 64 d partitions (bf16 SBUF -> DVE 2x multiply,
            # in place), then accumulated over 9 shifts by PE ident matmuls.
            def emit_E_dma(hp, gb):
                base = gb * 2048
                rps = []
                for s in range(K2):
                    rp = rppool.tile([128, 2048], bf16, tag="rp", name="rp")
                    for hl in range(2):
                        r = ROWOF(s, hl)
                        srcap = stage[hp][r : r + 1, base : base + 2048]
                        srcap = srcap.broadcast_to((64, 2048))
                        qeng = nc.sync
                        qeng.dma_start(rp[64 * hl : 64 * (hl + 1), :], srcap)
                    rps.append(rp)
                return rps

            def emit_E_cmp(hp, gb, rps):
                base = gb * 2048
                apss = [
                    ps_acc.tile([128, 512], f32, tag="accps", name="accps")
                    for _ in range(4)
                ]
                for s in range(K2):
                    nc.vector.tensor_tensor(
                        rps[s][:], rps[s][:], v_view(hp, s, base, 2048), MULT
                    )
                    for j in range(4):
                        nc.tensor.matmul(
                            apss[j][:],
                            ident_bf[:],
                            rps[s][:, j * 512 : (j + 1) * 512],
                            start=(s == 0),
                            stop=(s == K2 - 1),
                        )
                for j in range(4):
                    ncol = slice(base + j * 512, base + (j + 1) * 512)
                    nc.scalar.activation(acc[hp][:, ncol], apss[j][:], COPY)

            def emit_F_half(h):
                for mt in range(2):
                    for nt2 in (2 * h, 2 * h + 1):  # 1024-wide out stages
                        ps2 = [
                            ps_mm.tile([128, 512], f32, tag="mmps", name="projps")
                            for _ in range(2)
                        ]
                        ob = opool.tile([128, 1024], f32, tag="ob", name="ob")
                        for j in range(2):
                            ncol = slice(nt2 * 1024 + j * 512, nt2 * 1024 + (j + 1) * 512)
                            for kt in range(2):
                                nc.tensor.matmul(
                                    ps2[j][:],
                                    wpt_bf[kt][:, mt * 128 : (mt + 1) * 128],
                                    acc[kt][:, ncol],
                                    start=(kt == 0),
                                    stop=(kt == 1),
                                )
                            nc.scalar.activation(
                                ob[:, j * 512 : (j + 1) * 512], ps2[j][:], COPY
                            )
                        qeng = (nc.sync, nc.scalar)[nt2 % 2]
                        qeng.dma_start(
                            out_ext[
                                mt * 128 : (mt + 1) * 128,
                                nt2 * 1024 : (nt2 + 1) * 1024,
                            ],
                            ob[:],
                        )

            # ---- master schedule: per-half pipeline so broadcast DMA
            # spreads across the whole kernel ----
            emit_B([(0, 0, 128), (1, 128, 128), (2, 256, 128), (3, 384, 128)])
            emit_C(0, 0)
            emit_C(0, 1)
            emit_B([(4, 512, 128), (5, 640, 128), (6, 768, 128)])
            emit_D_z(0, 0)
            emit_D_wc(0, 0)
            rps00 = emit_E_dma(0, 0)
            emit_C(0, 2)
            emit_C(0, 3)
            emit_D_z(0, 1)
            emit_D_wc(0, 1)
            rps01 = emit_E_dma(0, 1)
            emit_C(1, 0)
            emit_E_cmp(0, 0, rps00)
            emit_C(1, 1)
            emit_D_z(1, 0)
            emit_D_wc(1, 0)
            emit_E_cmp(0, 1, rps01)
            rps10 = emit_E_dma(1, 0)
            emit_C(1, 2)
            emit_C(1, 3)
            emit_D_z(1, 1)
            emit_D_wc(1, 1)
            emit_E_cmp(1, 0, rps10)
            rps11 = emit_E_dma(1, 1)
            emit_F_half(0)
            emit_E_cmp(1, 1, rps11)
            emit_F_half(1)

    return nc


def kernel(**inputs):
    global LAST_EXEC_NS, LAST_TRACE_DIR
    x = np.asarray(inputs["x"], np.float32)
    w_qkv = np.asarray(inputs["w_qkv"], np.float32)
    w_kg = np.asarray(inputs["w_kg"], np.float32)
    b_kg = np.asarray(inputs["b_kg"], np.float32).reshape(-1)
    alpha = float(np.asarray(inputs["alpha"]))
    beta = float(np.asarray(inputs["beta"]))
    w_proj = np.asarray(inputs["w_proj"], np.float32)

    B = x.shape[0]
    # fold grouped kernel-generator through qkv weights; alpha folded in.
    # kg rows are stored permuted: row (hp*18 + ROWOF(s, hl)) holds head
    # (2*hp + hl), shift s.
    W_kgx = np.zeros((NH * K2, C), np.float32)
    for h in range(NH):
        W_kgx[h * K2 : (h + 1) * K2] = (
            w_kg[h * K2 : (h + 1) * K2] @ w_qkv[192 * h : 192 * (h + 1)]
        )
    VALID_ROWS = [ROWOF(s, hl) for s in range(K2) for hl in range(2)]
    W_kgp = np.zeros((128, C), np.float32)
    bkgp = np.zeros((128,), np.float32)
    for hp in range(2):
        for hl in range(2):
            for s in range(K2):
                h = 2 * hp + hl
                W_kgp[hp * 64 + ROWOF(s, hl)] = alpha * W_kgx[h * K2 + s]
                bkgp[hp * 64 + ROWOF(s, hl)] = alpha * b_kg[h * K2 + s]
    w_aug = np.concatenate([w_qkv, W_kgp], 0)  # (896, 256)
    wt = np.ascontiguousarray(w_aug.T)
    wpt = np.ascontiguousarray(w_proj.T)

    osel = np.zeros((128, 9 * RCO), np.float32)
    for s in range(K2):
        for hl in range(2):
            for d in range(D):
                osel[hl * D + d, s * RCO + ROWOF(s, hl)] = 1.0
    orep = np.zeros((RCO, 9 * 128), np.float32)
    for s in range(K2):
        for hl in range(2):
            orep[ROWOF(s, hl), s * 128 + hl * D : s * 128 + (hl + 1) * D] = 1.0
    # Z replication: every output row r (valid or not) gets the sum of the
    # valid rows of head r%2, so ln never sees 0 in unused rows
    ozrep = np.zeros((RCO, RCO), np.float32)
    for r in range(RCO):
        for r2 in VALID_ROWS:
            if r % 2 == r2 % 2:
                ozrep[r2, r] = 1.0
    ident = np.eye(128, dtype=np.float32)
    bkg = np.ascontiguousarray(bkgp.reshape(128, 1))

    nc = _build_graph(alpha, beta)
    if not nc.is_finalized():
        nc.finalize()

    W0 = 896 + 256 + 9 * RCO + 128
    W1 = 896 + 256 + RCO + 1
    cst0 = np.zeros((128, W0), np.float32)
    cst0[:, 0:896] = wt[0:128]
    cst0[:, 896:1152] = wpt[0:128]
    cst0[:, 1152 : 1152 + 9 * RCO] = osel
    cst0[:, 1152 + 9 * RCO :] = ident
    cst1 = np.zeros((128, W1), np.float32)
    cst1[:, 0:896] = wt[128:256]
    cst1[:, 896:1152] = wpt[128:256]
    cst1[0:RCO, 1152 : 1152 + RCO] = ozrep
    import ml_dtypes
    shared = dict(
        cst0=cst0.astype(ml_dtypes.bfloat16),
        cst1=cst1[:, 0 : W1 - 1].astype(ml_dtypes.bfloat16),
        bkgv=bkg,
    )
    in_maps = [
        dict(shared, x=np.ascontiguousarray(x[b].reshape(C, L))) for b in range(B)
    ]

    from concourse import bass_utils as _bu
    from concourse.bass_utils import run_bass_kernel_spmd

    trace = os.environ.get("KERNEL_TRACE", "0") == "1"
    tkw = {}
    if trace:
        import types

        try:
            import antenv.axon_hooks  # noqa: F401
        except ImportError:
            sys.path.insert(0, "/root/.axon_site")
            from trn_agent_boot.trn_boot import _ntff_profile_via_ctypes

            _mod = types.ModuleType("antenv.axon_hooks")
            _hook = _ntff_profile_via_ctypes("/opt/axon/libaxon_pjrt.so")
            _mod.get_axon_ntff_profile_hook = lambda: _hook
            _mod.set_axon_ntff_profile_hook = lambda h: None
            sys.modules["antenv.axon_hooks"] = _mod
        _bu.upload_artifacts = lambda tmpdir: "local://" + tmpdir
        import tempfile

        LAST_TRACE_DIR = tempfile.mkdtemp(prefix="ktrace_")
        tkw["tmpdir"] = LAST_TRACE_DIR
    res = run_bass_kernel_spmd(
        nc, in_maps, core_ids=list(range(NCORES)), trace=trace, **tkw
    )
    LAST_EXEC_NS = res.exec_time_ns
    out = np.stack(
        [np.asarray(res.results[b]["out"]).reshape(C, H_IMG, W_IMG) for b in range(B)]
    )
    return out.astype(np.float32)


# revision 51
# speedup vs baseline: 1.0085x; 1.0085x over previous
"""ACmix Trainium2 kernel: batch-parallel over 8 NeuronCores.

Per-core graph (one batch element, x: (256, 64, 64)):
  - qkv 1x1 conv as PE matmul; the grouped kernel-generator is folded through
    the qkv weights on the host (alpha * w_kg_g @ w_qkv_grouped) and rides as
    one extra 128-row M tile. q/k tiles are emitted first so the logits
    pipeline starts while v/kg tiles are still computing.
  - 3x3 window shifts are free-axis AP offsets into zero-padded (guard-band)
    k/v buffers; an odd-offset copy of k keeps DVE product reads pair-aligned
    (2x mode).
  - logits: DVE/GpSimd shifted elementwise products + PE selector matmuls
    reducing over d into (38, 512) PSUM tiles (coefficient rows are grouped
    by dj so image-edge fixups hit 32-aligned partition ranges).
  - softmax: edge columns of expS forced to exp(0)=1 (reference semantics
    for out-of-image shifts), Z via a replicating PE matmul, 1/Z via ActE
    ln + exp(-x + ln(beta)).
  - combined coefficients wc = invzb*expS + alpha*kg (wide in-place 2x DVE
    ops); wc edge columns zeroed so edge-wrapped pv terms vanish.
  - V-contraction: wc is bounced to DRAM and broadcast-read back replicated
    across each head's 64 d partitions (bf16 SBUF), multiplied by shifted v
    in place on DVE at 2x, and accumulated over the 9 shifts by PE identity
    matmuls in PSUM. This removes the PE replication matmuls and keeps the
    multiplies off the PSUM-read (1x) path.
  - final 1x1 proj matmul, f32 DMA out.

The whole kernel is software-pipelined at 2048-column granularity across the
two head-pairs so the broadcast DMA traffic spreads over the full runtime and
PE stays continuously busy (high p-state).
"""

import os
import sys

import numpy as np

sys.path.insert(0, "/opt/trn_rl_repo")

H_IMG = 64
W_IMG = 64
L = H_IMG * W_IMG  # 4096
C = 256
NH = 4
D = 64
K2 = 9
NCORES = 8
NT = 8  # 512-column chunks

LAST_EXEC_NS = None
LAST_TRACE_DIR = None

# shift table: s = 3*(di+1) + (dj+1), flat offset 64*di + dj
SHIFTS = [(di, dj) for di in (-1, 0, 1) for dj in (-1, 0, 1)]
# coefficient-row order: group shifts by dj so edge fixes hit contiguous,
# 32-aligned partition ranges: dj=-1 -> rows 0-5, dj=0 -> rows 6-11,
# dj=+1 -> rows 32-37. Rows 12-31 are unused (engines require 32-aligned
# partition bases for sub-tile views).
DJORD = {0: 0, 3: 1, 6: 2, 1: 3, 4: 4, 7: 5, 2: 6, 5: 7, 8: 8}
RCO = 38  # coefficient tile height


def ROWOF(s, hl):
    g = DJORD[s]
    return g * 2 + hl if g < 6 else 32 + (g - 6) * 2 + hl


def _build_graph(alpha, beta):
    import concourse.bacc as bacc
    import concourse.mybir as mybir
    from concourse import tile

    f32 = mybir.dt.float32
    bf16 = mybir.dt.bfloat16
    MULT = mybir.AluOpType.mult
    ADD = mybir.AluOpType.add
    EXP = mybir.ActivationFunctionType.Exp
    LOG = mybir.ActivationFunctionType.Ln
    COPY = mybir.ActivationFunctionType.Copy
    IDENT = mybir.ActivationFunctionType.Identity

    nc = bacc.Bacc(None, target_bir_lowering=False)

    x_ext = nc.declare_dram_parameter("x", [C, L], f32, isOutput=False)
    # cst0: wt0|wpt0|osel|ident, cst1: wt1|wpt1|ozrep(pad128)|bkg
    W0 = 896 + 256 + 9 * RCO + 128
    W1 = 896 + 256 + RCO + 1
    cst0_ext = nc.declare_dram_parameter("cst0", [128, W0], bf16, isOutput=False)
    cst1_ext = nc.declare_dram_parameter("cst1", [128, W1 - 1], bf16, isOutput=False)
    bkg_ext = nc.declare_dram_parameter("bkgv", [128, 1], f32, isOutput=False)
    out_ext = nc.declare_dram_parameter("out", [C, L], f32, isOutput=True)
    stage = [nc.dram_tensor(f"stage{h}", [RCO, L], bf16, kind="Internal") for h in range(2)]

    lnbeta = float(np.log(beta))

    with tile.TileContext(nc) as tc:
        with (
            tc.tile_pool(name="const", bufs=1) as cpool,
            tc.tile_pool(name="data", bufs=1) as dpool,
            tc.tile_pool(name="prod", bufs=4) as ppool,
            tc.tile_pool(name="lnz", bufs=4) as lnzpool,
            tc.tile_pool(name="rp", bufs=12) as rppool,
            tc.tile_pool(name="outsb", bufs=2) as opool,
            tc.tile_pool(name="ps_mm", bufs=2, space="PSUM") as ps_mm,
            tc.tile_pool(name="ps_lg", bufs=2, space="PSUM") as ps_lg,
            tc.tile_pool(name="ps_acc", bufs=4, space="PSUM") as ps_acc,
        ):
            # ---- constants (two fused blobs; bkg stays f32 via its own
            # small piece of cst1 read at f32) ----
            cst0 = cpool.tile([128, W0], bf16, tag="cst0")
            cst1b = cpool.tile([128, W1 - 1], bf16, tag="cst1b")
            bkg_sb = cpool.tile([128, 1], f32, tag="bkg")
            lnb_sb = cpool.tile([RCO, 1], f32, tag="lnb")
            nc.vector.memset(lnb_sb[:], lnbeta)
            nc.sync.dma_start(cst0[:], cst0_ext[:])
            nc.sync.dma_start(cst1b[:], cst1_ext[:])
            nc.sync.dma_start(bkg_sb[:], bkg_ext[:])
            wt_bf = [cst0[:, 0:896], cst1b[:, 0:896]]
            wpt_bf = [cst0[:, 896:1152], cst1b[:, 896:1152]]
            osel_bf = cst0[:, 1152 : 1152 + 9 * RCO]
            ident_bf = cst0[:, 1152 + 9 * RCO : 1152 + 9 * RCO + 128]
            ozrep_bf = cst1b[0:RCO, 1152 : 1152 + RCO]

            # ---- input (cast f32 -> bf16 during DMA) ----
            x_bf = [dpool.tile([128, L], bf16, tag=f"x{k}", name=f"x{k}") for k in range(2)]
            for nt in range(NT):
                ncol = slice(nt * 512, (nt + 1) * 512)
                for k in range(2):
                    nc.gpsimd.dma_start(x_bf[k][:, ncol], x_ext[128 * k : 128 * (k + 1), ncol])

            # ---- main SBUF tensors ----
            q_bf = [dpool.tile([128, L], bf16, tag=f"q{h}", name=f"q{h}") for h in range(2)]
            GUARD = 66
            FLATW = GUARD + L + GUARD  # 4228
            ke = [dpool.tile([128, FLATW], bf16, tag=f"ke{h}", name=f"ke{h}") for h in range(2)]
            ko = [dpool.tile([128, FLATW], bf16, tag=f"ko{h}", name=f"ko{h}") for h in range(2)]
            ve = [dpool.tile([128, FLATW], bf16, tag=f"ve{h}", name=f"ve{h}") for h in range(2)]
            kg = [dpool.tile([RCO, L], bf16, tag=f"kg{h}", name=f"kg{h}") for h in range(2)]
            expS = [dpool.tile([RCO, L], bf16, tag=f"expS{h}", name=f"expS{h}") for h in range(2)]

            # acc reuses q_bf: q is dead once its head-pair's products are done
            acc = q_bf

            # zero the guard bands
            for t in ko:
                nc.vector.memset(t[:, 0 : GUARD + 1], 0.0)
                nc.vector.memset(t[:, GUARD + 1 + L : FLATW], 0.0)
            for t in ke + ve:
                nc.vector.memset(t[:, 0:GUARD], 0.0)
                nc.vector.memset(t[:, GUARD + L : FLATW], 0.0)

            # ---- phase B: qkv + kernel-gen matmul, evictions ----
            # split: q/k tiles first (unblock phase C), v/kg tiles later
            def emit_B(tiles):
                for nt in range(NT):
                    ncol = slice(nt * 512, (nt + 1) * 512)
                    for mi, m0, msz in tiles:
                        ps = ps_mm.tile([msz, 512], f32, tag="mmps", name="qkvps", padded_shape=[128, 512])
                        for kt in range(2):
                            nc.tensor.matmul(
                                ps[:],
                                wt_bf[kt][:, m0 : m0 + msz],
                                x_bf[kt][:, ncol],
                                start=(kt == 0),
                                stop=(kt == 1),
                            )
                        if mi < 2:
                            nc.scalar.activation(q_bf[mi][:, ncol], ps[:], COPY)
                        elif mi < 4:
                            hp = mi - 2
                            dst = ke[hp][:, GUARD + 512 * nt : GUARD + 512 * (nt + 1)]
                            nc.scalar.activation(dst, ps[:], COPY)
                        elif mi < 6:
                            hp = mi - 4
                            dst = ve[hp][:, GUARD + 512 * nt : GUARD + 512 * (nt + 1)]
                            nc.scalar.activation(dst, ps[:], COPY)
                        else:
                            for hp in range(2):
                                nc.scalar.activation(
                                    kg[hp][:, ncol],
                                    ps[64 * hp : 64 * hp + RCO, :],
                                    IDENT,
                                    bias=bkg_sb[64 * hp : 64 * hp + RCO, :],
                                )
                    # odd-offset k copy, chunked so C can start early
                    if tiles[0][0] == 0:
                        for hp in range(2):
                            c0 = GUARD + 512 * nt
                            nc.vector.tensor_copy(
                                ko[hp][:, c0 + 1 : c0 + 513],
                                ke[hp][:, c0 : c0 + 512],
                            )

            def k_view(hp, s, l0, ncols):
                di, dj = SHIFTS[s]
                off = 64 * di + dj
                if (GUARD + off) % 2 == 0:
                    base = GUARD + off + l0
                    return ke[hp][:, base : base + ncols]
                base = GUARD + 1 + off + l0
                return ko[hp][:, base : base + ncols]

            def v_view(hp, s, l0, ncols):
                di, dj = SHIFTS[s]
                off = 64 * di + dj
                base = GUARD + off + l0
                return ve[hp][:, base : base + ncols]

            def coeff_edge_fix(t, value):
                # t: (18, L) tensor in dj-grouped row order; set invalid-dj
                # entries to `value`: rows 0:6 have dj=-1 (invalid at image
                # col 0), rows 12:18 have dj=+1 (invalid at image col 63)
                v3 = t[:].rearrange("p (r c) -> p r c", c=64)
                nc.vector.memset(v3[0:6, :, 0:1], value)
                nc.vector.memset(v3[32:38, :, 63:64], value)

            # ---- phase emitters (C/D/E/F), software-pipelined across the
            # two head-pairs so DMA-paced E overlaps compute-paced C/D ----
            def emit_C(hp, g):
                # logits products + selector matmuls + exp for one 1024-chunk
                gc = slice(g * 1024, (g + 1) * 1024)
                qv = q_bf[hp][:, gc]
                lgs = [ps_lg.tile([RCO, 512], f32, tag="lg", name="lg") for _ in range(2)]
                for s in range(K2):
                    prod = ppool.tile([128, 1024], bf16, tag="prod", name="prod")
                    kap = k_view(hp, s, g * 1024, 1024)
                    eng = nc.gpsimd if s in (1, 7) else nc.vector
                    eng.tensor_tensor(prod[:], qv, kap, MULT)
                    for j in range(2):
                        nc.tensor.matmul(
                            lgs[j][:],
                            osel_bf[:, s * RCO : (s + 1) * RCO],
                            prod[:, j * 512 : (j + 1) * 512],
                            start=(s == 0),
                            stop=(s == K2 - 1),
                        )
                for j in range(2):
                    ncol = slice(g * 1024 + j * 512, g * 1024 + (j + 1) * 512)
                    nc.scalar.activation(expS[hp][:, ncol], lgs[j][:], EXP)

            lnz_tiles = {}

            def emit_D_z(hp, gb):
                # dj-edge logits are garbage (wrapped rows); reference has
                # logit=0 there -> exp=1 must enter Z (rows of this half)
                v3 = expS[hp][:].rearrange("p (r c) -> p r c", c=64)
                r0, r1 = gb * 32, (gb + 1) * 32
                nc.gpsimd.memset(v3[0:6, r0:r1, 0:1], 1.0)
                nc.gpsimd.memset(v3[32:38, r0:r1, 63:64], 1.0)
                lz = lnzpool.tile([RCO, 2048], bf16, tag="lnz", name="lnz")
                lnz_tiles[(hp, gb)] = lz
                for k4 in range(4):
                    nt = 4 * gb + k4
                    ncol = slice(nt * 512, (nt + 1) * 512)
                    zr = ps_mm.tile([RCO, 512], f32, tag="mmps", name="zr", padded_shape=[128, 512])
                    nc.tensor.matmul(zr[:], ozrep_bf[:], expS[hp][:, ncol])
                    nc.scalar.activation(lz[:, k4 * 512 : (k4 + 1) * 512], zr[:], LOG)

            def emit_D_wc(hp, gb):
                # invzb = beta * exp(-ln Z); wc = invzb * expS + kg (in expS)
                hc = slice(gb * 2048, (gb + 1) * 2048)
                lz = lnz_tiles.pop((hp, gb))
                iz = lnzpool.tile([RCO, 2048], bf16, tag="lnz", name="invz")
                nc.scalar.activation(iz[:], lz[:], EXP, scale=-1.0, bias=lnb_sb[:])
                nc.vector.tensor_tensor(
                    expS[hp][:, hc], iz[:], expS[hp][:, hc], MULT
                )
                nc.vector.tensor_tensor(
                    expS[hp][:, hc], expS[hp][:, hc], kg[hp][:, hc], ADD
                )
                # edge-wrapped pv terms must vanish (rows of this half only)
                v3 = expS[hp][:].rearrange("p (r c) -> p r c", c=64)
                r0, r1 = gb * 32, (gb + 1) * 32
                nc.gpsimd.memset(v3[0:6, r0:r1, 0:1], 0.0)
                nc.gpsimd.memset(v3[32:38, r0:r1, 63:64], 0.0)
                # bounce this half of wc to DRAM for phase-E broadcast reads
                nc.scalar.dma_start(stage[hp][:, hc], expS[hp][:, hc])

            # phase E: wc broadcast-read back from DRAM replicated across
            # each head's 64 d partitions (bf16 SBUF -> DVE 2x multiply,
            # in place), then accumulated over 9 shifts by PE ident matmuls.
            def emit_E_dma(hp, gb):
                base = gb * 2048
                rps = []
                for s in range(K2):
                    rp = rppool.tile([128, 2048], bf16, tag="rp", name="rp")
                    for hl in range(2):
                        r = ROWOF(s, hl)
                        srcap = stage[hp][r : r + 1, base : base + 2048]
                        srcap = srcap.broadcast_to((64, 2048))
                        qeng = nc.sync
                        qeng.dma_start(rp[64 * hl : 64 * (hl + 1), :], srcap)
                    rps.append(rp)
                return rps

            def emit_E_cmp(hp, gb, rps):
                base = gb * 2048
                apss = [
                    ps_acc.tile([128, 512], f32, tag="accps", name="accps")
                    for _ in range(4)
                ]
                for s in range(K2):
                    nc.vector.tensor_tensor(
                        rps[s][:], rps[s][:], v_view(hp, s, base, 2048), MULT
                    )
                    for j in range(4):
                        nc.tensor.matmul(
                            apss[j][:],
                            ident_bf[:],
                            rps[s][:, j * 512 : (j + 1) * 512],
                            start=(s == 0),
                            stop=(s == K2 - 1),
                        )
                for j in range(4):
                    ncol = slice(base + j * 512, base + (j + 1) * 512)
                    nc.scalar.activation(acc[hp][:, ncol], apss[j][:], COPY)

            def emit_F_half(h):
                for mt in range(2):
                    for nt2 in (2 * h, 2 * h + 1):  # 1024-wide out stages
                        ps2 = [
                            ps_mm.tile([128, 512], f32, tag="mmps", name="projps")
                            for _ in range(2)
                        ]
                        ob = opool.tile([128, 1024], f32, tag="ob", name="ob")
                        for j in range(2):
                            ncol = slice(nt2 * 1024 + j * 512, nt2 * 1024 + (j + 1) * 512)
                            for kt in range(2):
                                nc.tensor.matmul(
                                    ps2[j][:],
                                    wpt_bf[kt][:, mt * 128 : (mt + 1) * 128],
                                    acc[kt][:, ncol],
                                    start=(kt == 0),
                                    stop=(kt == 1),
                                )
                            nc.scalar.activation(
                                ob[:, j * 512 : (j + 1) * 512], ps2[j][:], COPY
                            )
                        qeng = (nc.sync, nc.scalar)[nt2 % 2]
                        qeng.dma_start(
                            out_ext[
                                mt * 128 : (mt + 1) * 128,
                                nt2 * 1024 : (nt2 + 1) * 1024,
                            ],
                            ob[:],
                        )

            # ---- master schedule: per-half pipeline so broadcast DMA
            # spreads across the whole kernel ----
            emit_B([(0, 0, 128), (1, 128, 128), (2, 256, 128), (3, 384, 128)])
            emit_C(0, 0)
            emit_C(0, 1)
            emit_B([(4, 512, 128), (5, 640, 128), (6, 768, 128)])
            emit_D_z(0, 0)
            emit_D_wc(0, 0)
            rps00 = emit_E_dma(0, 0)
            emit_C(0, 2)
            emit_C(0, 3)
            emit_D_z(0, 1)
            emit_D_wc(0, 1)
            rps01 = emit_E_dma(0, 1)
            emit_C(1, 0)
            emit_E_cmp(0, 0, rps00)
            emit_C(1, 1)
            emit_D_z(1, 0)
            emit_D_wc(1, 0)
            emit_E_cmp(0, 1, rps01)
            rps10 = emit_E_dma(1, 0)
            emit_C(1, 2)
            emit_C(1, 3)
            emit_D_z(1, 1)
            emit_D_wc(1, 1)
            emit_E_cmp(1, 0, rps10)
            rps11 = emit_E_dma(1, 1)
            emit_F_half(0)
            emit_E_cmp(1, 1, rps11)
            emit_F_half(1)

    return nc


def kernel(**inputs):
    global LAST_EXEC_NS, LAST_TRACE_DIR
    x = np.asarray(inputs["x"], np.float32)
    w_qkv = np.asarray(inputs["w_qkv"], np.float32)
    w_kg = np.asarray(inputs["w_kg"], np.float32)
    b_kg = np.asarray(inputs["b_kg"], np.float32).reshape(-1)
    alpha = float(np.asarray(inputs["alpha"]))
    beta = float(np.asarray(inputs["beta"]))
    w_proj = np.asarray(inputs["w_proj"], np.float32)

    B = x.shape[0]
    # fold grouped kernel-generator through qkv weights; alpha folded in.
    # kg rows are stored permuted: row (hp*18 + ROWOF(s, hl)) holds head
    # (2*hp + hl), shift s.
    W_kgx = np.zeros((NH * K2, C), np.float32)
    for h in range(NH):
        W_kgx[h * K2 : (h + 1) * K2] = (
            w_kg[h * K2 : (h + 1) * K2] @ w_qkv[192 * h : 192 * (h + 1)]
        )
    VALID_ROWS = [ROWOF(s, hl) for s in range(K2) for hl in range(2)]
    W_kgp = np.zeros((128, C), np.float32)
    bkgp = np.zeros((128,), np.float32)
    for hp in range(2):
        for hl in range(2):
            for s in range(K2):
                h = 2 * hp + hl
                W_kgp[hp * 64 + ROWOF(s, hl)] = alpha * W_kgx[h * K2 + s]
                bkgp[hp * 64 + ROWOF(s, hl)] = alpha * b_kg[h * K2 + s]
    w_aug = np.concatenate([w_qkv, W_kgp], 0)  # (896, 256)
    wt = np.ascontiguousarray(w_aug.T)
    wpt = np.ascontiguousarray(w_proj.T)

    osel = np.zeros((128, 9 * RCO), np.float32)
    for s in range(K2):
        for hl in range(2):
            for d in range(D):
                osel[hl * D + d, s * RCO + ROWOF(s, hl)] = 1.0
    orep = np.zeros((RCO, 9 * 128), np.float32)
    for s in range(K2):
        for hl in range(2):
            orep[ROWOF(s, hl), s * 128 + hl * D : s * 128 + (hl + 1) * D] = 1.0
    # Z replication: every output row r (valid or not) gets the sum of the
    # valid rows of head r%2, so ln never sees 0 in unused rows
    ozrep = np.zeros((RCO, RCO), np.float32)
    for r in range(RCO):
        for r2 in VALID_ROWS:
            if r % 2 == r2 % 2:
                ozrep[r2, r] = 1.0
    ident = np.eye(128, dtype=np.float32)
    bkg = np.ascontiguousarray(bkgp.reshape(128, 1))

    nc = _build_graph(alpha, beta)
    if not nc.is_finalized():
        nc.finalize()

    W0 = 896 + 256 + 9 * RCO + 128
    W1 = 896 + 256 + RCO + 1
    cst0 = np.zeros((128, W0), np.float32)
    cst0[:, 0:896] = wt[0:128]
    cst0[:, 896:1152] = wpt[0:128]
    cst0[:, 1152 : 1152 + 9 * RCO] = osel
    cst0[:, 1152 + 9 * RCO :] = ident
    cst1 = np.zeros((128, W1), np.float32)
    cst1[:, 0:896] = wt[128:256]
    cst1[:, 896:1152] = wpt[128:256]
    cst1[0:RCO, 1152 : 1152 + RCO] = ozrep
    import ml_dtypes
    shared = dict(
        cst0=cst0.astype(ml_dtypes.bfloat16),
        cst1=cst1[:, 0 : W1 - 1].astype(ml_dtypes.bfloat16),
        bkgv=bkg,
    )
    in_maps = [
        dict(shared, x=np.ascontiguousarray(x[b].reshape(C, L))) for b in range(B)
    ]

    from concourse import bass_utils as _bu
    from concourse.bass_utils import run_bass_kernel_spmd

    trace = os.environ.get("KERNEL_TRACE", "0") == "1"
    tkw = {}
    if trace:
        import types

        try:
            import antenv.axon_hooks  # noqa: F401
        except ImportError:
            sys.path.insert(0, "/root/.axon_site")
            from trn_agent_boot.trn_boot import _ntff_profile_via_ctypes

            _mod = types.ModuleType("antenv.axon_hooks")
            _hook = _ntff_profile_via_ctypes("/opt/axon/libaxon_pjrt.so")
            _mod.get_axon_ntff_profile_hook = lambda: _hook
            _mod.set_axon_ntff_profile_hook = lambda h: None
            sys.modules["antenv.axon_hooks"] = _mod
        _bu.upload_artifacts = lambda tmpdir: "local://" + tmpdir
        import tempfile

        LAST_TRACE_DIR = tempfile.mkdtemp(prefix="ktrace_")
        tkw["tmpdir"] = LAST_TRACE_DIR
    res = run_bass_kernel_spmd(
        nc, in_maps, core_ids=list(range(NCORES)), trace=trace, **tkw
    )
    LAST_EXEC_NS = res.exec_time_ns
    out = np.stack(
        [np.asarray(res.results[b]["out"]).reshape(C, H_IMG, W_IMG) for b in range(B)]
    )
    return out.astype(np.float32)


# revision 52
# speedup vs baseline: 1.0598x; 1.0509x over previous
"""ACmix Trainium2 kernel: batch-parallel over 8 NeuronCores.

Per-core graph (one batch element, x: (256, 64, 64)):
  - qkv 1x1 conv as PE matmul; the grouped kernel-generator is folded through
    the qkv weights on the host (alpha * w_kg_g @ w_qkv_grouped) and rides as
    one extra 128-row M tile. q/k tiles are emitted first so the logits
    pipeline starts while v/kg tiles are still computing.
  - 3x3 window shifts are free-axis AP offsets into zero-padded (guard-band)
    k/v buffers; an odd-offset copy of k keeps DVE product reads pair-aligned
    (2x mode).
  - logits: DVE/GpSimd shifted elementwise products + PE selector matmuls
    reducing over d into (38, 512) PSUM tiles (coefficient rows are grouped
    by dj so image-edge fixups hit 32-aligned partition ranges).
  - softmax: edge columns of expS forced to exp(0)=1 (reference semantics
    for out-of-image shifts), Z via a replicating PE matmul, 1/Z via ActE
    ln + exp(-x + ln(beta)).
  - combined coefficients wc = invzb*expS + alpha*kg (wide in-place 2x DVE
    ops); wc edge columns zeroed so edge-wrapped pv terms vanish.
  - V-contraction: wc is bounced to DRAM and broadcast-read back replicated
    across each head's 64 d partitions (bf16 SBUF), multiplied by shifted v
    in place on DVE at 2x, and accumulated over the 9 shifts by PE identity
    matmuls in PSUM. This removes the PE replication matmuls and keeps the
    multiplies off the PSUM-read (1x) path.
  - final 1x1 proj matmul, f32 DMA out.

The whole kernel is software-pipelined at 2048-column granularity across the
two head-pairs so the broadcast DMA traffic spreads over the full runtime and
PE stays continuously busy (high p-state).
"""

import os
import sys

import numpy as np

sys.path.insert(0, "/opt/trn_rl_repo")

H_IMG = 64
W_IMG = 64
L = H_IMG * W_IMG  # 4096
C = 256
NH = 4
D = 64
K2 = 9
NCORES = 8
NT = 8  # 512-column chunks

LAST_EXEC_NS = None
LAST_TRACE_DIR = None

# shift table: s = 3*(di+1) + (dj+1), flat offset 64*di + dj
SHIFTS = [(di, dj) for di in (-1, 0, 1) for dj in (-1, 0, 1)]
# coefficient-row order: group shifts by dj so edge fixes hit contiguous,
# 32-aligned partition ranges: dj=-1 -> rows 0-5, dj=0 -> rows 6-11,
# dj=+1 -> rows 32-37. Rows 12-31 are unused (engines require 32-aligned
# partition bases for sub-tile views).
DJORD = {0: 0, 3: 1, 6: 2, 1: 3, 4: 4, 7: 5, 2: 6, 5: 7, 8: 8}
RCO = 38  # coefficient tile height


def ROWOF(s, hl):
    g = DJORD[s]
    return g * 2 + hl if g < 6 else 32 + (g - 6) * 2 + hl


def _build_graph(alpha, beta):
    import concourse.bacc as bacc
    import concourse.mybir as mybir
    from concourse import tile

    f32 = mybir.dt.float32
    bf16 = mybir.dt.bfloat16
    MULT = mybir.AluOpType.mult
    ADD = mybir.AluOpType.add
    EXP = mybir.ActivationFunctionType.Exp
    LOG = mybir.ActivationFunctionType.Ln
    COPY = mybir.ActivationFunctionType.Copy
    IDENT = mybir.ActivationFunctionType.Identity

    nc = bacc.Bacc(None, target_bir_lowering=False)

    x_ext = nc.declare_dram_parameter("x", [C, L], f32, isOutput=False)
    # cst0: wt0|wpt0|osel|ident, cst1: wt1|wpt1|ozrep(pad128)|bkg
    W0 = 896 + 256 + 9 * RCO + 128
    W1 = 896 + 256 + RCO + 1
    cst0_ext = nc.declare_dram_parameter("cst0", [128, W0], bf16, isOutput=False)
    cst1_ext = nc.declare_dram_parameter("cst1", [128, W1 - 1], bf16, isOutput=False)
    bkg_ext = nc.declare_dram_parameter("bkgv", [128, 1], f32, isOutput=False)
    out_ext = nc.declare_dram_parameter("out", [C, L], f32, isOutput=True)
    stage = [nc.dram_tensor(f"stage{h}", [RCO, L], bf16, kind="Internal") for h in range(2)]

    lnbeta = float(np.log(beta))

    with tile.TileContext(nc) as tc:
        with (
            tc.tile_pool(name="const", bufs=1) as cpool,
            tc.tile_pool(name="data", bufs=1) as dpool,
            tc.tile_pool(name="prod", bufs=4) as ppool,
            tc.tile_pool(name="lnz", bufs=4) as lnzpool,
            tc.tile_pool(name="rp", bufs=12) as rppool,
            tc.tile_pool(name="outsb", bufs=2) as opool,
            tc.tile_pool(name="ps_mm", bufs=2, space="PSUM") as ps_mm,
            tc.tile_pool(name="ps_lg", bufs=2, space="PSUM") as ps_lg,
            tc.tile_pool(name="ps_acc", bufs=4, space="PSUM") as ps_acc,
        ):
            # ---- constants (two fused blobs; bkg stays f32 via its own
            # small piece of cst1 read at f32) ----
            cst0 = cpool.tile([128, W0], bf16, tag="cst0")
            cst1b = cpool.tile([128, W1 - 1], bf16, tag="cst1b")
            bkg_sb = cpool.tile([128, 1], f32, tag="bkg")
            lnb_sb = cpool.tile([RCO, 1], f32, tag="lnb")
            nc.vector.memset(lnb_sb[:], lnbeta)
            nc.sync.dma_start(cst0[:], cst0_ext[:])
            nc.sync.dma_start(cst1b[:], cst1_ext[:])
            nc.sync.dma_start(bkg_sb[:], bkg_ext[:])
            wt_bf = [cst0[:, 0:896], cst1b[:, 0:896]]
            wpt_bf = [cst0[:, 896:1152], cst1b[:, 896:1152]]
            osel_bf = cst0[:, 1152 : 1152 + 9 * RCO]
            ident_bf = cst0[:, 1152 + 9 * RCO : 1152 + 9 * RCO + 128]
            ozrep_bf = cst1b[0:RCO, 1152 : 1152 + RCO]

            # ---- input (cast f32 -> bf16 during DMA) ----
            x_bf = [dpool.tile([128, L], bf16, tag=f"x{k}", name=f"x{k}") for k in range(2)]
            for nt in range(NT):
                ncol = slice(nt * 512, (nt + 1) * 512)
                for k in range(2):
                    nc.gpsimd.dma_start(x_bf[k][:, ncol], x_ext[128 * k : 128 * (k + 1), ncol])

            # ---- main SBUF tensors ----
            q_bf = [dpool.tile([128, L], bf16, tag=f"q{h}", name=f"q{h}") for h in range(2)]
            GUARD = 66
            FLATW = GUARD + L + GUARD  # 4228
            ke = [dpool.tile([128, FLATW], bf16, tag=f"ke{h}", name=f"ke{h}") for h in range(2)]
            ko = [dpool.tile([128, FLATW], bf16, tag=f"ko{h}", name=f"ko{h}") for h in range(2)]
            ve = [dpool.tile([128, FLATW], bf16, tag=f"ve{h}", name=f"ve{h}") for h in range(2)]
            kg = [dpool.tile([RCO, L], bf16, tag=f"kg{h}", name=f"kg{h}") for h in range(2)]
            expS = [dpool.tile([RCO, L], bf16, tag=f"expS{h}", name=f"expS{h}") for h in range(2)]

            # acc reuses q_bf: q is dead once its head-pair's products are done
            acc = q_bf

            # zero the guard bands
            for t in ko:
                nc.vector.memset(t[:, 0 : GUARD + 1], 0.0)
                nc.vector.memset(t[:, GUARD + 1 + L : FLATW], 0.0)
            for t in ke + ve:
                nc.vector.memset(t[:, 0:GUARD], 0.0)
                nc.vector.memset(t[:, GUARD + L : FLATW], 0.0)

            # ---- phase B: qkv + kernel-gen matmul, evictions ----
            # split: q/k tiles first (unblock phase C), v/kg tiles later
            def emit_B(tiles):
                for nt in range(NT):
                    ncol = slice(nt * 512, (nt + 1) * 512)
                    for mi, m0, msz in tiles:
                        ps = ps_mm.tile([msz, 512], f32, tag="mmps", name="qkvps", padded_shape=[128, 512])
                        for kt in range(2):
                            nc.tensor.matmul(
                                ps[:],
                                wt_bf[kt][:, m0 : m0 + msz],
                                x_bf[kt][:, ncol],
                                start=(kt == 0),
                                stop=(kt == 1),
                            )
                        if mi < 2:
                            nc.scalar.activation(q_bf[mi][:, ncol], ps[:], COPY)
                        elif mi < 4:
                            hp = mi - 2
                            dst = ke[hp][:, GUARD + 512 * nt : GUARD + 512 * (nt + 1)]
                            nc.scalar.activation(dst, ps[:], COPY)
                        elif mi < 6:
                            hp = mi - 4
                            dst = ve[hp][:, GUARD + 512 * nt : GUARD + 512 * (nt + 1)]
                            nc.scalar.activation(dst, ps[:], COPY)
                        else:
                            for hp in range(2):
                                nc.scalar.activation(
                                    kg[hp][:, ncol],
                                    ps[64 * hp : 64 * hp + RCO, :],
                                    IDENT,
                                    bias=bkg_sb[64 * hp : 64 * hp + RCO, :],
                                )
                    # odd-offset k copy, chunked so C can start early
                    if tiles[0][0] == 0:
                        for hp in range(2):
                            c0 = GUARD + 512 * nt
                            nc.vector.tensor_copy(
                                ko[hp][:, c0 + 1 : c0 + 513],
                                ke[hp][:, c0 : c0 + 512],
                            )

            def k_view(hp, s, l0, ncols):
                di, dj = SHIFTS[s]
                off = 64 * di + dj
                if (GUARD + off) % 2 == 0:
                    base = GUARD + off + l0
                    return ke[hp][:, base : base + ncols]
                base = GUARD + 1 + off + l0
                return ko[hp][:, base : base + ncols]

            def v_view(hp, s, l0, ncols):
                di, dj = SHIFTS[s]
                off = 64 * di + dj
                base = GUARD + off + l0
                return ve[hp][:, base : base + ncols]

            def coeff_edge_fix(t, value):
                # t: (18, L) tensor in dj-grouped row order; set invalid-dj
                # entries to `value`: rows 0:6 have dj=-1 (invalid at image
                # col 0), rows 12:18 have dj=+1 (invalid at image col 63)
                v3 = t[:].rearrange("p (r c) -> p r c", c=64)
                nc.vector.memset(v3[0:6, :, 0:1], value)
                nc.vector.memset(v3[32:38, :, 63:64], value)

            # ---- phase emitters (C/D/E/F), software-pipelined across the
            # two head-pairs so DMA-paced E overlaps compute-paced C/D ----
            def emit_C(hp, g):
                # logits products + selector matmuls + exp for one 1024-chunk
                gc = slice(g * 1024, (g + 1) * 1024)
                qv = q_bf[hp][:, gc]
                lgs = [ps_lg.tile([RCO, 512], f32, tag="lg", name="lg") for _ in range(2)]
                for s in range(K2):
                    prod = ppool.tile([128, 1024], bf16, tag="prod", name="prod")
                    kap = k_view(hp, s, g * 1024, 1024)
                    eng = nc.gpsimd if s in (1, 7) else nc.vector
                    eng.tensor_tensor(prod[:], qv, kap, MULT)
                    for j in range(2):
                        nc.tensor.matmul(
                            lgs[j][:],
                            osel_bf[:, s * RCO : (s + 1) * RCO],
                            prod[:, j * 512 : (j + 1) * 512],
                            start=(s == 0),
                            stop=(s == K2 - 1),
                        )
                for j in range(2):
                    ncol = slice(g * 1024 + j * 512, g * 1024 + (j + 1) * 512)
                    nc.scalar.activation(expS[hp][:, ncol], lgs[j][:], EXP)

            lnz_tiles = {}

            def emit_D_z(hp, gb):
                # dj-edge logits are garbage (wrapped rows); reference has
                # logit=0 there -> exp=1 must enter Z (rows of this half)
                v3 = expS[hp][:].rearrange("p (r c) -> p r c", c=64)
                r0, r1 = gb * 32, (gb + 1) * 32
                nc.gpsimd.memset(v3[0:6, r0:r1, 0:1], 1.0)
                nc.gpsimd.memset(v3[32:38, r0:r1, 63:64], 1.0)
                lz = lnzpool.tile([RCO, 2048], bf16, tag="lnz", name="lnz")
                lnz_tiles[(hp, gb)] = lz
                for k4 in range(4):
                    nt = 4 * gb + k4
                    ncol = slice(nt * 512, (nt + 1) * 512)
                    zr = ps_mm.tile([RCO, 512], f32, tag="mmps", name="zr", padded_shape=[128, 512])
                    nc.tensor.matmul(zr[:], ozrep_bf[:], expS[hp][:, ncol])
                    nc.scalar.activation(lz[:, k4 * 512 : (k4 + 1) * 512], zr[:], LOG)

            def emit_D_wc(hp, gb):
                # invzb = beta * exp(-ln Z); wc = invzb * expS + kg (in expS)
                hc = slice(gb * 2048, (gb + 1) * 2048)
                lz = lnz_tiles.pop((hp, gb))
                iz = lnzpool.tile([RCO, 2048], bf16, tag="lnz", name="invz")
                nc.scalar.activation(iz[:], lz[:], EXP, scale=-1.0, bias=lnb_sb[:])
                nc.vector.tensor_tensor(
                    expS[hp][:, hc], iz[:], expS[hp][:, hc], MULT
                )
                nc.vector.tensor_tensor(
                    expS[hp][:, hc], expS[hp][:, hc], kg[hp][:, hc], ADD
                )
                # edge-wrapped pv terms must vanish (rows of this half only)
                v3 = expS[hp][:].rearrange("p (r c) -> p r c", c=64)
                r0, r1 = gb * 32, (gb + 1) * 32
                nc.gpsimd.memset(v3[0:6, r0:r1, 0:1], 0.0)
                nc.gpsimd.memset(v3[32:38, r0:r1, 63:64], 0.0)
                # bounce this half of wc to DRAM for phase-E broadcast reads
                nc.scalar.dma_start(stage[hp][:, hc], expS[hp][:, hc])

            # phase E: wc broadcast-read back from DRAM replicated across
            # each head's 64 d partitions (bf16 SBUF -> DVE 2x multiply,
            # in place), then accumulated over 9 shifts by PE ident matmuls.
            def emit_E_dma(hp, gb):
                base = gb * 2048
                rps = []
                for s in range(K2):
                    rp = rppool.tile([128, 2048], bf16, tag="rp", name="rp")
                    for hl in range(2):
                        r = ROWOF(s, hl)
                        srcap = stage[hp][r : r + 1, base : base + 2048]
                        srcap = srcap.broadcast_to((64, 2048))
                        qeng = nc.sync
                        qeng.dma_start(rp[64 * hl : 64 * (hl + 1), :], srcap)
                    rps.append(rp)
                return rps

            def emit_E_cmp(hp, gb, rps):
                base = gb * 2048
                apss = [
                    ps_acc.tile([128, 512], f32, tag="accps", name="accps")
                    for _ in range(4)
                ]
                for s in range(K2):
                    nc.vector.tensor_tensor(
                        rps[s][:], rps[s][:], v_view(hp, s, base, 2048), MULT
                    )
                    for j in range(4):
                        nc.tensor.matmul(
                            apss[j][:],
                            ident_bf[:],
                            rps[s][:, j * 512 : (j + 1) * 512],
                            start=(s == 0),
                            stop=(s == K2 - 1),
                        )
                for j in range(4):
                    ncol = slice(base + j * 512, base + (j + 1) * 512)
                    nc.scalar.activation(acc[hp][:, ncol], apss[j][:], COPY)

            def emit_F_half(h):
                for mt in range(2):
                    for nt2 in (2 * h, 2 * h + 1):  # 1024-wide out stages
                        ps2 = [
                            ps_mm.tile([128, 512], f32, tag="mmps", name="projps")
                            for _ in range(2)
                        ]
                        ob = opool.tile([128, 1024], f32, tag="ob", name="ob")
                        for j in range(2):
                            ncol = slice(nt2 * 1024 + j * 512, nt2 * 1024 + (j + 1) * 512)
                            for kt in range(2):
                                nc.tensor.matmul(
                                    ps2[j][:],
                                    wpt_bf[kt][:, mt * 128 : (mt + 1) * 128],
                                    acc[kt][:, ncol],
                                    start=(kt == 0),
                                    stop=(kt == 1),
                                )
                            nc.scalar.activation(
                                ob[:, j * 512 : (j + 1) * 512], ps2[j][:], COPY
                            )
                        qeng = (nc.sync, nc.scalar)[nt2 % 2]
                        qeng.dma_start(
                            out_ext[
                                mt * 128 : (mt + 1) * 128,
                                nt2 * 1024 : (nt2 + 1) * 1024,
                            ],
                            ob[:],
                        )

            # ---- master schedule: per-half pipeline so broadcast DMA
            # spreads across the whole kernel ----
            emit_B([(0, 0, 128), (1, 128, 128), (2, 256, 128), (3, 384, 128)])
            emit_C(0, 0)
            emit_C(0, 1)
            emit_B([(6, 768, 128)])
            emit_D_z(0, 0)
            emit_D_wc(0, 0)
            rps00 = emit_E_dma(0, 0)
            emit_B([(4, 512, 128), (5, 640, 128)])
            emit_C(0, 2)
            emit_C(0, 3)
            emit_D_z(0, 1)
            emit_D_wc(0, 1)
            rps01 = emit_E_dma(0, 1)
            emit_C(1, 0)
            emit_E_cmp(0, 0, rps00)
            emit_C(1, 1)
            emit_D_z(1, 0)
            emit_D_wc(1, 0)
            emit_E_cmp(0, 1, rps01)
            rps10 = emit_E_dma(1, 0)
            emit_C(1, 2)
            emit_C(1, 3)
            emit_D_z(1, 1)
            emit_D_wc(1, 1)
            emit_E_cmp(1, 0, rps10)
            rps11 = emit_E_dma(1, 1)
            emit_F_half(0)
            emit_E_cmp(1, 1, rps11)
            emit_F_half(1)

    return nc


def kernel(**inputs):
    global LAST_EXEC_NS, LAST_TRACE_DIR
    x = np.asarray(inputs["x"], np.float32)
    w_qkv = np.asarray(inputs["w_qkv"], np.float32)
    w_kg = np.asarray(inputs["w_kg"], np.float32)
    b_kg = np.asarray(inputs["b_kg"], np.float32).reshape(-1)
    alpha = float(np.asarray(inputs["alpha"]))
    beta = float(np.asarray(inputs["beta"]))
    w_proj = np.asarray(inputs["w_proj"], np.float32)

    B = x.shape[0]
    # fold grouped kernel-generator through qkv weights; alpha folded in.
    # kg rows are stored permuted: row (hp*18 + ROWOF(s, hl)) holds head
    # (2*hp + hl), shift s.
    W_kgx = np.zeros((NH * K2, C), np.float32)
    for h in range(NH):
        W_kgx[h * K2 : (h + 1) * K2] = (
            w_kg[h * K2 : (h + 1) * K2] @ w_qkv[192 * h : 192 * (h + 1)]
        )
    VALID_ROWS = [ROWOF(s, hl) for s in range(K2) for hl in range(2)]
    W_kgp = np.zeros((128, C), np.float32)
    bkgp = np.zeros((128,), np.float32)
    for hp in range(2):
        for hl in range(2):
            for s in range(K2):
                h = 2 * hp + hl
                W_kgp[hp * 64 + ROWOF(s, hl)] = alpha * W_kgx[h * K2 + s]
                bkgp[hp * 64 + ROWOF(s, hl)] = alpha * b_kg[h * K2 + s]
    w_aug = np.concatenate([w_qkv, W_kgp], 0)  # (896, 256)
    wt = np.ascontiguousarray(w_aug.T)
    wpt = np.ascontiguousarray(w_proj.T)

    osel = np.zeros((128, 9 * RCO), np.float32)
    for s in range(K2):
        for hl in range(2):
            for d in range(D):
                osel[hl * D + d, s * RCO + ROWOF(s, hl)] = 1.0
    orep = np.zeros((RCO, 9 * 128), np.float32)
    for s in range(K2):
        for hl in range(2):
            orep[ROWOF(s, hl), s * 128 + hl * D : s * 128 + (hl + 1) * D] = 1.0
    # Z replication: every output row r (valid or not) gets the sum of the
    # valid rows of head r%2, so ln never sees 0 in unused rows
    ozrep = np.zeros((RCO, RCO), np.float32)
    for r in range(RCO):
        for r2 in VALID_ROWS:
            if r % 2 == r2 % 2:
                ozrep[r2, r] = 1.0
    ident = np.eye(128, dtype=np.float32)
    bkg = np.ascontiguousarray(bkgp.reshape(128, 1))

    nc = _build_graph(alpha, beta)
    if not nc.is_finalized():
        nc.finalize()

    W0 = 896 + 256 + 9 * RCO + 128
    W1 = 896 + 256 + RCO + 1
    cst0 = np.zeros((128, W0), np.float32)
    cst0[:, 0:896] = wt[0:128]
    cst0[:, 896:1152] = wpt[0:128]
    cst0[:, 1152 : 1152 + 9 * RCO] = osel
    cst0[:, 1152 + 9 * RCO :] = ident
    cst1 = np.zeros((128, W1), np.float32)
    cst1[:, 0:896] = wt[128:256]
    cst1[:, 896:1152] = wpt[128:256]
    cst1[0:RCO, 1152 : 1152 + RCO] = ozrep
    import ml_dtypes
    shared = dict(
        cst0=cst0.astype(ml_dtypes.bfloat16),
        cst1=cst1[:, 0 : W1 - 1].astype(ml_dtypes.bfloat16),
        bkgv=bkg,
    )
    in_maps = [
        dict(shared, x=np.ascontiguousarray(x[b].reshape(C, L))) for b in range(B)
    ]

    from concourse import bass_utils as _bu
    from concourse.bass_utils import run_bass_kernel_spmd

    trace = os.environ.get("KERNEL_TRACE", "0") == "1"
    tkw = {}
    if trace:
        import types

        try:
            import antenv.axon_hooks  # noqa: F401
        except ImportError:
            sys.path.insert(0, "/root/.axon_site")
            from trn_agent_boot.trn_boot import _ntff_profile_via_ctypes

            _mod = types.ModuleType("antenv.axon_hooks")
            _hook = _ntff_profile_via_ctypes("/opt/axon/libaxon_pjrt.so")
            _mod.get_axon_ntff_profile_hook = lambda: _hook
            _mod.set_axon_ntff_profile_hook = lambda h: None
            sys.modules["antenv.axon_hooks"] = _mod
        _bu.upload_artifacts = lambda tmpdir: "local://" + tmpdir
        import tempfile

        LAST_TRACE_DIR = tempfile.mkdtemp(prefix="ktrace_")
        tkw["tmpdir"] = LAST_TRACE_DIR
    res = run_bass_kernel_spmd(
        nc, in_maps, core_ids=list(range(NCORES)), trace=trace, **tkw
    )
    LAST_EXEC_NS = res.exec_time_ns
    out = np.stack(
        [np.asarray(res.results[b]["out"]).reshape(C, H_IMG, W_IMG) for b in range(B)]
    )
    return out.astype(np.float32)
